# revision 4
# baseline (speedup 1.0000x reference)
"""Bass/Trainium2 kernel for nn_EquivSetGNN3 (gnn_message_passing).

Math (reference): x = relu(x@W_in+b_in); x0 = x
  2 layers of: Xe = segsum_E((x@W1+b1)[V]); Xev = cat(x[V], Xe[E])@W2+b2
               Xv = segsum_V(Xev); x = relu((0.5*Xv + 0.5*x0)@W3 + b3)

Algebraic restructuring (avoids all [nnz, C] feature materialization):
  Xe = (segsum_E x[V]) @ W1 + deg_E (x) b1
  Xv = deg_V (*) (x @ W2a) + (segsum_V Xe[E]) @ W2b + deg_V (x) b2
where W2a = W2[:C], W2b = W2[C:].

Segment sums run as dma_gather of 256B f16 rows + one-hot matmuls on the
TensorEngine (128 incidences -> <=128 segments per chunk, accumulated in
PSUM per 128-segment superchunk). Gathers are batched into large calls
(grouped superchunks) to amortize the ~1us fixed SWDGE cost per call; the
one-hot P matrices are generated on the fly by the DVE (is_equal against
an iota row, broadcast APs), not streamed from DRAM.

Sharding: nodes and edges split 8 ways (graph parallel); x and Xe are
AllGathered (Shared-output fast path) between phases; weights replicated.
"""
import numpy as np

import concourse.bacc as bacc
import concourse.mybir as mybir
import concourse.tile as tile
from concourse.bass_utils import run_bass_kernel_spmd

f32 = mybir.dt.float32
f32r = mybir.dt.float32r
f16 = mybir.dt.float16
i16 = mybir.dt.int16

N = 50000
M = 25000
NNZ = 800000
C = 128
R = 8
NO = N // R          # 6250 nodes per core
EO = M // R          # 3125 edges per core
SPLIT = 32768        # int16 positive range limit for gather indices
SCA = (EO + 127) // 128   # 25 edge superchunks per core
SCB = (NO + 127) // 128   # 49 node superchunks per core
GA = 2               # phase-A superchunks per gather group
GB = 4               # phase-B superchunks per gather group
N_LAYERS = 2
ALPHA = 0.5

GROUPS_A = [list(range(g, min(g + GA, SCA))) for g in range(0, SCA, GA)]
GROUPS_B = [list(range(g, min(g + GB, SCB))) for g in range(0, SCB, GB)]

_cache = {}


def _wrap_idx(flat):
    """[L] int -> [128, L//16] int16 (idx i at partition i%16, col i//16;
    replicated 8x across partition groups for the 8 gpsimd cores)."""
    w = flat.reshape(-1, 16).T.astype(np.int16)
    return np.ascontiguousarray(np.tile(w, (8, 1)))


def _wrap_rel(flat):
    """[L] int -> [128, L//128] f16 (value i at partition i%128, col i//128)."""
    return np.ascontiguousarray(flat.reshape(-1, 128).T.astype(np.float16))


def _pad_to(arr, n, val):
    out = np.full(n, val, dtype=arr.dtype)
    out[: len(arr)] = arr
    return out


def _prepare(V, E):
    """Host-side preprocessing: sorted/sharded/padded gather index+rel arrays.

    Stream layout (per core) matches the kernel's grouped gathers:
      phase A, group g: [bucketA chunks of each SC in g][bucketB chunks ...]
      phase B, group g: [chunks of each SC in g]
    """
    # ---- phase A: incidences sorted by E (edge-major) ----
    oA = np.argsort(E, kind="stable")
    Va, Ea = V[oA], E[oA]
    e0 = np.arange(R)[:, None] * EO + np.arange(SCA)[None, :] * 128  # [R,SCA]
    e1 = np.minimum(e0 + 128, (np.arange(R)[:, None] + 1) * EO)
    lo = np.searchsorted(Ea, e0.ravel()).reshape(R, SCA)
    hi = np.searchsorted(Ea, e1.ravel()).reshape(R, SCA)

    cntA = np.zeros((R, SCA), np.int64)
    cntB = np.zeros((R, SCA), np.int64)
    for r in range(R):
        for s in range(SCA):
            seg = Va[lo[r, s] : hi[r, s]]
            nb = int((seg >= SPLIT).sum())
            cntB[r, s] = nb
            cntA[r, s] = len(seg) - nb
    nchA = (-(-cntA.max(0) // 128)).astype(np.int64)  # [SCA] cross-core max
    nchB = (-(-cntB.max(0) // 128)).astype(np.int64)
    assert (nchA + nchB > 0).all()

    # ---- phase B: incidences sorted by V (node-major) ----
    oB = np.argsort(V, kind="stable")
    Vb, Eb = V[oB], E[oB]
    v0 = np.arange(R)[:, None] * NO + np.arange(SCB)[None, :] * 128
    v1 = np.minimum(v0 + 128, (np.arange(R)[:, None] + 1) * NO)
    lo2 = np.searchsorted(Vb, v0.ravel()).reshape(R, SCB)
    hi2 = np.searchsorted(Vb, v1.ravel()).reshape(R, SCB)
    cnt2 = hi2 - lo2
    nch2 = (-(-cnt2.max(0) // 128)).astype(np.int64)  # [SCB]
    assert (nch2 > 0).all()

    meta = {
        "nchA": nchA.tolist(),
        "nchB": nchB.tolist(),
        "nch2": nch2.tolist(),
    }

    degE = np.bincount(E, minlength=M).astype(np.float32)
    degV = np.bincount(V, minlength=N).astype(np.float32)

    per_core = []
    for r in range(R):
        idxA_parts, relA_parts = [], []
        for grp in GROUPS_A:
            segs = {}
            for s in grp:
                seg_v = Va[lo[r, s] : hi[r, s]]
                seg_e = Ea[lo[r, s] : hi[r, s]] - e0[r, s]
                segs[s] = (seg_v, seg_e, seg_v >= SPLIT)
            for s in grp:  # bucket A (V < SPLIT)
                seg_v, seg_e, mB = segs[s]
                la = int(nchA[s]) * 128
                idxA_parts.append(_pad_to(seg_v[~mB], la, 0))
                relA_parts.append(_pad_to(seg_e[~mB], la, -1))
            for s in grp:  # bucket B (V >= SPLIT, rebased)
                seg_v, seg_e, mB = segs[s]
                lb = int(nchB[s]) * 128
                idxA_parts.append(_pad_to(seg_v[mB] - SPLIT, lb, 0))
                relA_parts.append(_pad_to(seg_e[mB], lb, -1))
        idxA = np.concatenate(idxA_parts)
        relA = np.concatenate(relA_parts)

        idxB_parts, relB_parts = [], []
        for grp in GROUPS_B:
            for s in grp:
                seg_e = Eb[lo2[r, s] : hi2[r, s]]
                seg_v = Vb[lo2[r, s] : hi2[r, s]] - v0[r, s]
                lb = int(nch2[s]) * 128
                idxB_parts.append(_pad_to(seg_e, lb, 0))
                relB_parts.append(_pad_to(seg_v, lb, -1))
        idxB = np.concatenate(idxB_parts)
        relB = np.concatenate(relB_parts)

        per_core.append(
            {
                "idxA": _wrap_idx(idxA),
                "relA": _wrap_rel(relA),
                "idxB": _wrap_idx(idxB),
                "relB": _wrap_rel(relB),
                "degE": degE[r * EO : (r + 1) * EO],
                "degV": degV[r * NO : (r + 1) * NO],
            }
        )
    return meta, per_core


def _build(meta):
    nchA = meta["nchA"]
    nchB = meta["nchB"]
    nch2 = meta["nch2"]
    LA = sum(a + b for a, b in zip(nchA, nchB)) * 128
    LB = sum(nch2) * 128
    NCHA_TOT = LA // 128
    NCHB_TOT = LB // 128

    nc = bacc.Bacc("TRN2", target_bir_lowering=False, debug=False, num_devices=R,
                   num_swdge_queues=4)

    # ---- kernel I/O ----
    xsh = nc.declare_dram_parameter("xsh", [NO, C], f32, isOutput=False)
    w_in = nc.declare_dram_parameter("w_in", [C, C], f32, isOutput=False)
    w1 = nc.declare_dram_parameter("w1", [C, C], f32, isOutput=False)
    w2a = nc.declare_dram_parameter("w2a", [C, C], f32, isOutput=False)
    w2b = nc.declare_dram_parameter("w2b", [C, C], f32, isOutput=False)
    w3h = nc.declare_dram_parameter("w3h", [C, C], f32, isOutput=False)
    b_in = nc.declare_dram_parameter("b_in", [C, 1], f32, isOutput=False)
    b2d = nc.declare_dram_parameter("b2", [C, 1], f32, isOutput=False)
    b3d = nc.declare_dram_parameter("b3", [C, 1], f32, isOutput=False)
    b1e_d = nc.declare_dram_parameter("b1e", [C, EO], f32, isOutput=False)
    dvrep_d = nc.declare_dram_parameter("dvrep", [C, NO], f32, isOutput=False)
    idxA_d = nc.declare_dram_parameter("idxA", [128, LA // 16], i16, isOutput=False)
    relA_d = nc.declare_dram_parameter("relA", [128, NCHA_TOT], f16, isOutput=False)
    idxB_d = nc.declare_dram_parameter("idxB", [128, LB // 16], i16, isOutput=False)
    relB_d = nc.declare_dram_parameter("relB", [128, NCHB_TOT], f16, isOutput=False)
    xout = nc.declare_dram_parameter("xout", [NO, C], f32, isOutput=True)

    # ---- internal DRAM ----
    agx_in = [nc.dram_tensor(f"agx_in{l}", [NO, C], f16) for l in range(N_LAYERS)]
    x_full = [nc.dram_tensor(f"x_full{l}", [N, C], f16, addr_space="Shared")
              for l in range(N_LAYERS)]
    agxe_in = [nc.dram_tensor(f"agxe_in{l}", [EO, C], f16) for l in range(N_LAYERS)]
    xe_full = [nc.dram_tensor(f"xe_full{l}", [M, C], f16, addr_space="Shared")
               for l in range(N_LAYERS)]

    rg = [list(range(R))]
    qrr = [0]

    def next_q():
        q = qrr[0]
        qrr[0] = (q + 1) % 4
        return q

    with tile.TileContext(nc) as tc:
        with (
            tc.tile_pool(name="const", bufs=1) as cp,
            tc.tile_pool(name="work", bufs=2) as wp,
            tc.tile_pool(name="ptiles", bufs=3) as pp,
            tc.tile_pool(name="psA", bufs=2, space="PSUM") as psA,
            tc.tile_pool(name="psB", bufs=2, space="PSUM") as psB,
            tc.tile_pool(name="psC", bufs=2, space="PSUM") as psC,
            tc.tile_pool(name="psD", bufs=2, space="PSUM") as psD,
        ):
            # ---------- persistent tiles ----------
            W_IN = cp.tile([C, C], f32)
            W1 = cp.tile([C, C], f32)
            W2A = cp.tile([C, C], f32)
            W2B = cp.tile([C, C], f32)
            W3H = cp.tile([C, C], f32)
            BIN = cp.tile([C, 1], f32)
            B2 = cp.tile([C, 1], f32)
            B3 = cp.tile([C, 1], f32)
            IDXA = cp.tile([128, LA // 16], i16)
            IDXB = cp.tile([128, LB // 16], i16)
            RELA = cp.tile([128, NCHA_TOT], f16)
            RELB = cp.tile([128, NCHB_TOT], f16)
            XFM = cp.tile([C, NO], f32)
            X0B = cp.tile([C, NO], f32)
            IOTAF = cp.tile([128, 128], f32)
            IOTA16 = cp.tile([128, 128], f16)
            PIDX = cp.tile([128, 1], f32)
            IDENT = cp.tile([128, 128], f32)

            for t, d in [
                (W_IN, w_in), (W1, w1), (W2A, w2a), (W2B, w2b), (W3H, w3h),
                (BIN, b_in), (B2, b2d), (B3, b3d),
                (IDXA, idxA_d), (IDXB, idxB_d), (RELA, relA_d), (RELB, relB_d),
            ]:
                nc.sync.dma_start(t[:], d[:])

            nc.gpsimd.iota(IOTAF[:], [[1, 128]], channel_multiplier=0,
                           allow_small_or_imprecise_dtypes=True)
            nc.gpsimd.iota(PIDX[:], [[1, 1]], channel_multiplier=1,
                           allow_small_or_imprecise_dtypes=True)
            nc.vector.tensor_scalar(IDENT[:], IOTAF[:], PIDX[:], None,
                                    mybir.AluOpType.is_equal)
            nc.vector.tensor_copy(IOTA16[:], IOTAF[:])

            def pgen(rel_tile, c0, nch):
                """P[p, c, j] = (rel[p, c0+c] == j), one DVE op."""
                P = pp.tile([128, max(nch, 1), 128], f16, tag="P")
                nc.vector.tensor_tensor(
                    P[:, :nch, :],
                    rel_tile[:, c0 : c0 + nch].unsqueeze(2)
                        .broadcast_to([128, nch, 128]),
                    IOTA16[:].unsqueeze(1).broadcast_to([128, nch, 128]),
                    mybir.AluOpType.is_equal)
                return P

            # ---------- prologue: x = relu(x @ W_in + b_in) ----------
            for n0 in range(0, NO, 512):
                nn = min(512, NO - n0)
                nsc = (nn + 127) // 128
                xin = wp.tile([128, 512], f32, tag="xin")
                for si in range(nsc):
                    ns = min(128, nn - si * 128)
                    nc.sync.dma_start(xin[:ns, si * 128 : si * 128 + C],
                                      xsh[n0 + si * 128 : n0 + si * 128 + ns, :])
                ptr = psD.tile([128, 512], f32, tag="tr")
                for si in range(nsc):
                    ns = min(128, nn - si * 128)
                    nc.tensor.transpose(ptr[:, si * 128 : si * 128 + ns],
                                        xin[:ns, si * 128 : si * 128 + C],
                                        IDENT[:ns, :ns])
                xT = wp.tile([C, 512], f32, tag="xT")
                nc.vector.tensor_copy(xT[:, :nn], ptr[:, :nn])
                pmm = psB.tile([C, 512], f32, tag="mmA")
                nc.tensor.matmul(pmm[:, :nn], W_IN[:], xT[:, :nn])
                nc.scalar.activation(XFM[:, n0 : n0 + nn], pmm[:, :nn],
                                     mybir.ActivationFunctionType.Relu,
                                     bias=BIN[:, :1])
                # X0B = x0 + deg_V * b2  (phase-B restart + bias, pre-folded)
                dvt = wp.tile([C, 512], f32, tag="dvt")
                nc.sync.dma_start(dvt[:, :nn], dvrep_d[:, n0 : n0 + nn])
                tmp = wp.tile([C, 512], f32, tag="xdeg")
                nc.vector.tensor_scalar(tmp[:, :nn], dvt[:, :nn], B2[:, :1],
                                        None, mybir.AluOpType.mult)
                nc.vector.tensor_tensor(X0B[:, n0 : n0 + nn], tmp[:, :nn],
                                        XFM[:, n0 : n0 + nn],
                                        mybir.AluOpType.add)
                # row-major f16 copy (gather source)
                ptr2 = psD.tile([128, 512], f32, tag="tr")
                xrm = wp.tile([128, 512], f16, tag="xrm")
                for si in range(nsc):
                    ns = min(128, nn - si * 128)
                    blk = slice(si * 128, si * 128 + C)
                    nc.tensor.transpose(
                        ptr2[:ns, blk],
                        XFM[:, n0 + si * 128 : n0 + si * 128 + ns], IDENT[:])
                    nc.vector.tensor_copy(xrm[:ns, blk], ptr2[:ns, blk])
                    nc.sync.dma_start(
                        agx_in[0][n0 + si * 128 : n0 + si * 128 + ns, :],
                        xrm[:ns, blk])
            nc.gpsimd.collective_compute(
                "AllGather", mybir.AluOpType.bypass, replica_groups=rg,
                ins=[agx_in[0][:]], outs=[x_full[0][:]],
            )

            # ---------- conv layers ----------
            for l in range(N_LAYERS):
                xf = x_full[l]
                # ---- phase A: Xe = (segsum_E x[V]) @ W1 + b1*degE ----
                colA = 0
                slotA = 0
                for grp in GROUPS_A:
                    e0g = grp[0] * 128
                    ne_g = min(128 * len(grp), EO - e0g)
                    na_g = sum(nchA[s] for s in grp)
                    nb_g = sum(nchB[s] for s in grp)
                    ntot = na_g + nb_g
                    gt = wp.tile([128, ntot, C], f16, tag="gath")
                    if na_g > 0:
                        nc.gpsimd.dma_gather(
                            out_ap=gt[:, :na_g, :], in_ap=xf[:],
                            idxs_ap=IDXA[:, slotA // 16 :
                                         (slotA + na_g * 128) // 16],
                            num_idxs=na_g * 128, num_idxs_reg=na_g * 128,
                            elem_size=C, single_packet=False,
                            queue_num=next_q(),
                        )
                    if nb_g > 0:
                        sl = slotA + na_g * 128
                        nc.gpsimd.dma_gather(
                            out_ap=gt[:, na_g:ntot, :], in_ap=xf[SPLIT:, :],
                            idxs_ap=IDXA[:, sl // 16 : (sl + nb_g * 128) // 16],
                            num_idxs=nb_g * 128, num_idxs_reg=nb_g * 128,
                            elem_size=C, single_packet=False,
                            queue_num=next_q(),
                        )
                    slotA += ntot * 128
                    Pg = pgen(RELA, colA, ntot)
                    ps = psA.tile([C, 512], f32, tag="seg")
                    aoff = 0
                    boff = na_g
                    for si, s in enumerate(grp):
                        seq = (list(range(aoff, aoff + nchA[s]))
                               + list(range(boff, boff + nchB[s])))
                        aoff += nchA[s]
                        boff += nchB[s]
                        dst = ps[:, si * 128 : (si + 1) * 128]
                        for j, cc in enumerate(seq):
                            nc.tensor.matmul(dst, gt[:, cc, :], Pg[:, cc, :],
                                             start=(j == 0),
                                             stop=(j == len(seq) - 1))
                    colA += ntot
                    gsb = wp.tile([C, 512], f32, tag="gsb")
                    nc.vector.tensor_copy(gsb[:, :ne_g], ps[:, :ne_g])
                    b1t = wp.tile([C, 512], f32, tag="dvt")
                    nc.sync.dma_start(b1t[:, :ne_g], b1e_d[:, e0g : e0g + ne_g])
                    pxe = psB.tile([C, 512], f32, tag="mmA")
                    nc.tensor.matmul(pxe[:, :ne_g], W1[:], gsb[:, :ne_g])
                    xesb = wp.tile([C, 512], f32, tag="xesb")
                    nc.vector.tensor_tensor(xesb[:, :ne_g], pxe[:, :ne_g],
                                            b1t[:, :ne_g], mybir.AluOpType.add)
                    ptr = psD.tile([128, 512], f32, tag="tr")
                    xerm = wp.tile([128, 512], f16, tag="xrm")
                    for si in range(len(grp)):
                        ns = min(128, ne_g - si * 128)
                        if ns <= 0:
                            break
                        blk = slice(si * 128, si * 128 + C)
                        nc.tensor.transpose(
                            ptr[:ns, blk],
                            xesb[:, si * 128 : si * 128 + ns], IDENT[:])
                        nc.vector.tensor_copy(xerm[:ns, blk], ptr[:ns, blk])
                        nc.sync.dma_start(
                            agxe_in[l][e0g + si * 128 : e0g + si * 128 + ns, :],
                            xerm[:ns, blk])
                nc.gpsimd.collective_compute(
                    "AllGather", mybir.AluOpType.bypass, replica_groups=rg,
                    ins=[agxe_in[l][:]], outs=[xe_full[l][:]],
                )

                # ---- phase B ----
                last = l == N_LAYERS - 1
                colB = 0
                slotB = 0
                for grp in GROUPS_B:
                    n0g = grp[0] * 128
                    nn_g = min(128 * len(grp), NO - n0g)
                    nch_g = sum(nch2[s] for s in grp)
                    gt = wp.tile([128, nch_g, C], f16, tag="gath")
                    nc.gpsimd.dma_gather(
                        out_ap=gt[:, :, :], in_ap=xe_full[l][:],
                        idxs_ap=IDXB[:, slotB // 16 :
                                     (slotB + nch_g * 128) // 16],
                        num_idxs=nch_g * 128, num_idxs_reg=nch_g * 128,
                        elem_size=C, single_packet=False, queue_num=next_q(),
                    )
                    slotB += nch_g * 128
                    Pg = pgen(RELB, colB, nch_g)
                    ps = psA.tile([C, 512], f32, tag="seg")
                    off = 0
                    for si, s in enumerate(grp):
                        dst = ps[:, si * 128 : (si + 1) * 128]
                        nchs = nch2[s]
                        for j in range(nchs):
                            nc.tensor.matmul(dst, gt[:, off + j, :],
                                             Pg[:, off + j, :],
                                             start=(j == 0),
                                             stop=(j == nchs - 1))
                        off += nchs
                    colB += nch_g
                    ysb = wp.tile([C, 512], f32, tag="gsb")
                    nc.vector.tensor_copy(ysb[:, :nn_g], ps[:, :nn_g])
                    dvt = wp.tile([C, 512], f32, tag="dvt")
                    nc.sync.dma_start(dvt[:, :nn_g], dvrep_d[:, n0g : n0g + nn_g])
                    xdeg = wp.tile([C, 512], f32, tag="xdeg")
                    nc.vector.tensor_tensor(xdeg[:, :nn_g],
                                            XFM[:, n0g : n0g + nn_g],
                                            dvt[:, :nn_g],
                                            mybir.AluOpType.mult)
                    pab = psB.tile([C, 512], f32, tag="mmA")
                    nc.tensor.matmul(pab[:, :nn_g], W2A[:], xdeg[:, :nn_g],
                                     start=True, stop=False)
                    nc.tensor.matmul(pab[:, :nn_g], W2B[:], ysb[:, :nn_g],
                                     start=False, stop=True)
                    xmid = wp.tile([C, 512], f32, tag="xesb")
                    nc.vector.tensor_tensor(xmid[:, :nn_g], pab[:, :nn_g],
                                            X0B[:, n0g : n0g + nn_g],
                                            mybir.AluOpType.add)
                    pc = psC.tile([C, 512], f32, tag="out")
                    nc.tensor.matmul(pc[:, :nn_g], W3H[:], xmid[:, :nn_g])
                    nc.scalar.activation(XFM[:, n0g : n0g + nn_g], pc[:, :nn_g],
                                         mybir.ActivationFunctionType.Relu,
                                         bias=B3[:, :1])
                    ptr = psD.tile([128, 512], f32, tag="tr")
                    xrm = wp.tile([128, 512], f32 if last else f16,
                                  tag="xrmf" if last else "xrm")
                    dstt = xout if last else agx_in[l + 1]
                    for si in range(len(grp)):
                        ns = min(128, nn_g - si * 128)
                        if ns <= 0:
                            break
                        blk = slice(si * 128, si * 128 + C)
                        nc.tensor.transpose(
                            ptr[:ns, blk],
                            XFM[:, n0g + si * 128 : n0g + si * 128 + ns],
                            IDENT[:])
                        nc.vector.tensor_copy(xrm[:ns, blk], ptr[:ns, blk])
                        nc.sync.dma_start(
                            dstt[n0g + si * 128 : n0g + si * 128 + ns, :],
                            xrm[:ns, blk])
                if not last:
                    nc.gpsimd.collective_compute(
                        "AllGather", mybir.AluOpType.bypass, replica_groups=rg,
                        ins=[agx_in[l + 1][:]], outs=[x_full[l + 1][:]],
                    )
    nc.compile()
    return nc


def _get_program(V, E):
    key = (hash(V.tobytes()), hash(E.tobytes()))
    if key not in _cache:
        meta, per_core = _prepare(V, E)
        nc = _build(meta)
        _cache[key] = (nc, per_core)
    return _cache[key]


def run(trace=False, trace_kwargs=None, **inputs):
    x = np.ascontiguousarray(np.asarray(inputs["x"], dtype=np.float32))
    V = np.asarray(inputs["V"]).astype(np.int64)
    E = np.asarray(inputs["E"]).astype(np.int64)
    W_in = np.ascontiguousarray(np.asarray(inputs["W_in"], np.float32))
    b_in = np.asarray(inputs["b_in"], np.float32).reshape(C, 1)
    W1 = np.ascontiguousarray(np.asarray(inputs["W1"], np.float32))
    b1 = np.asarray(inputs["b1"], np.float32).reshape(C)
    W2 = np.asarray(inputs["W2"], np.float32)
    b2 = np.asarray(inputs["b2"], np.float32).reshape(C, 1)
    W3 = np.asarray(inputs["W3"], np.float32)
    b3 = np.asarray(inputs["b3"], np.float32).reshape(C, 1)
    W2a = np.ascontiguousarray(W2[:C])
    W2b = np.ascontiguousarray(W2[C:])
    W3h = np.ascontiguousarray((1.0 - ALPHA) * W3)
    # note: (1-a)*Xv + a*x0 = (1-a)*(Xv + x0) since a = 0.5

    nc, per_core = _get_program(V, E)

    in_maps = []
    for r in range(R):
        pc = per_core[r]
        b1e = np.ascontiguousarray(np.outer(b1, pc["degE"]).astype(np.float32))
        dvrep = np.ascontiguousarray(
            np.broadcast_to(pc["degV"], (C, NO)).astype(np.float32))
        in_maps.append({
            "xsh": x[r * NO : (r + 1) * NO],
            "w_in": W_in, "w1": W1, "w2a": W2a, "w2b": W2b, "w3h": W3h,
            "b_in": b_in, "b2": b2, "b3": b3,
            "b1e": b1e, "dvrep": dvrep,
            "idxA": pc["idxA"], "relA": pc["relA"],
            "idxB": pc["idxB"], "relB": pc["relB"],
        })
    res = run_bass_kernel_spmd(nc, in_maps, list(range(R)), trace=trace,
                               **(trace_kwargs or {}))
    out = np.concatenate([res.results[r]["xout"] for r in range(R)], axis=0)
    return out, res


def kernel(**inputs):
    out, _ = run(**inputs)
    return out


# revision 6
# speedup vs baseline: 1.5451x; 1.5451x over previous
"""Bass/Trainium2 kernel for nn_EquivSetGNN3 (gnn_message_passing).

Math (reference): x = relu(x@W_in+b_in); x0 = x
  2 layers of: Xe = segsum_E((x@W1+b1)[V]); Xev = cat(x[V], Xe[E])@W2+b2
               Xv = segsum_V(Xev); x = relu((0.5*Xv + 0.5*x0)@W3 + b3)

Algebraic restructuring (avoids all [nnz, C] feature materialization):
  Xe = (segsum_E x[V]) @ W1 + deg_E (x) b1
  Xv = deg_V (*) (x @ W2a) + (segsum_V Xe[E]) @ W2b + deg_V (x) b2
where W2a = W2[:C], W2b = W2[C:].

Segment sums run as dma_gather of 256B f16 rows + one-hot matmuls on the
TensorEngine (128 incidences -> <=128 segments per chunk, accumulated in
PSUM per 128-segment superchunk). Gathers are batched into large calls
(grouped superchunks) to amortize the ~1us fixed SWDGE cost per call; the
one-hot P matrices are generated on the fly by the DVE (is_equal against
an iota row, broadcast APs), not streamed from DRAM.

Sharding: nodes and edges split 8 ways (graph parallel); x and Xe are
AllGathered (Shared-output fast path) between phases; weights replicated.
"""
import numpy as np

import concourse.bacc as bacc
import concourse.mybir as mybir
import concourse.tile as tile
from concourse.bass_utils import run_bass_kernel_spmd

f32 = mybir.dt.float32
f32r = mybir.dt.float32r
f16 = mybir.dt.float16
i16 = mybir.dt.int16

N = 50000
M = 25000
NNZ = 800000
C = 128
R = 8
NO = N // R          # 6250 nodes per core
EO = M // R          # 3125 edges per core
SPLIT = 32768        # int16 positive range limit for gather indices
SCA = (EO + 127) // 128   # 25 edge superchunks per core
SCB = (NO + 127) // 128   # 49 node superchunks per core
GA = 2               # phase-A superchunks per gather group
GB = 4               # phase-B superchunks per gather group
N_LAYERS = 2
ALPHA = 0.5

GROUPS_A = [list(range(g, min(g + GA, SCA))) for g in range(0, SCA, GA)]
GROUPS_B = [list(range(g, min(g + GB, SCB))) for g in range(0, SCB, GB)]

_cache = {}


def _wrap_idx(flat):
    """[L] int -> [128, L//16] int16 (idx i at partition i%16, col i//16;
    replicated 8x across partition groups for the 8 gpsimd cores)."""
    w = flat.reshape(-1, 16).T.astype(np.int16)
    return np.ascontiguousarray(np.tile(w, (8, 1)))


def _wrap_rel(flat):
    """[L] int -> [128, L//128] f16 (value i at partition i%128, col i//128)."""
    return np.ascontiguousarray(flat.reshape(-1, 128).T.astype(np.float16))


def _pad_to(arr, n, val):
    out = np.full(n, val, dtype=arr.dtype)
    out[: len(arr)] = arr
    return out


def _prepare(V, E):
    """Host-side preprocessing: sorted/sharded/padded gather index+rel arrays.

    Stream layout (per core) matches the kernel's grouped gathers:
      phase A, group g: [bucketA chunks of each SC in g][bucketB chunks ...]
      phase B, group g: [chunks of each SC in g]
    """
    # ---- phase A: incidences sorted by E (edge-major) ----
    oA = np.argsort(E, kind="stable")
    Va, Ea = V[oA], E[oA]
    e0 = np.arange(R)[:, None] * EO + np.arange(SCA)[None, :] * 128  # [R,SCA]
    e1 = np.minimum(e0 + 128, (np.arange(R)[:, None] + 1) * EO)
    lo = np.searchsorted(Ea, e0.ravel()).reshape(R, SCA)
    hi = np.searchsorted(Ea, e1.ravel()).reshape(R, SCA)

    cntA = np.zeros((R, SCA), np.int64)
    cntB = np.zeros((R, SCA), np.int64)
    for r in range(R):
        for s in range(SCA):
            seg = Va[lo[r, s] : hi[r, s]]
            nb = int((seg >= SPLIT).sum())
            cntB[r, s] = nb
            cntA[r, s] = len(seg) - nb
    nchA = (-(-cntA.max(0) // 128)).astype(np.int64)  # [SCA] cross-core max
    nchB = (-(-cntB.max(0) // 128)).astype(np.int64)
    assert (nchA + nchB > 0).all()

    # ---- phase B: incidences sorted by V (node-major) ----
    oB = np.argsort(V, kind="stable")
    Vb, Eb = V[oB], E[oB]
    v0 = np.arange(R)[:, None] * NO + np.arange(SCB)[None, :] * 128
    v1 = np.minimum(v0 + 128, (np.arange(R)[:, None] + 1) * NO)
    lo2 = np.searchsorted(Vb, v0.ravel()).reshape(R, SCB)
    hi2 = np.searchsorted(Vb, v1.ravel()).reshape(R, SCB)
    cnt2 = hi2 - lo2
    nch2 = (-(-cnt2.max(0) // 128)).astype(np.int64)  # [SCB]
    assert (nch2 > 0).all()

    meta = {
        "nchA": nchA.tolist(),
        "nchB": nchB.tolist(),
        "nch2": nch2.tolist(),
    }

    degE = np.bincount(E, minlength=M).astype(np.float32)
    degV = np.bincount(V, minlength=N).astype(np.float32)

    per_core = []
    for r in range(R):
        idxA_parts, relA_parts = [], []
        for grp in GROUPS_A:
            segs = {}
            for s in grp:
                seg_v = Va[lo[r, s] : hi[r, s]]
                seg_e = Ea[lo[r, s] : hi[r, s]] - e0[r, s]
                segs[s] = (seg_v, seg_e, seg_v >= SPLIT)
            for s in grp:  # bucket A (V < SPLIT)
                seg_v, seg_e, mB = segs[s]
                la = int(nchA[s]) * 128
                idxA_parts.append(_pad_to(seg_v[~mB], la, 0))
                relA_parts.append(_pad_to(seg_e[~mB], la, -1))
            for s in grp:  # bucket B (V >= SPLIT, rebased)
                seg_v, seg_e, mB = segs[s]
                lb = int(nchB[s]) * 128
                idxA_parts.append(_pad_to(seg_v[mB] - SPLIT, lb, 0))
                relA_parts.append(_pad_to(seg_e[mB], lb, -1))
        idxA = np.concatenate(idxA_parts)
        relA = np.concatenate(relA_parts)

        idxB_parts, relB_parts = [], []
        for grp in GROUPS_B:
            for s in grp:
                seg_e = Eb[lo2[r, s] : hi2[r, s]]
                seg_v = Vb[lo2[r, s] : hi2[r, s]] - v0[r, s]
                lb = int(nch2[s]) * 128
                idxB_parts.append(_pad_to(seg_e, lb, 0))
                relB_parts.append(_pad_to(seg_v, lb, -1))
        idxB = np.concatenate(idxB_parts)
        relB = np.concatenate(relB_parts)

        per_core.append(
            {
                "idxA": _wrap_idx(idxA),
                "relA": _wrap_rel(relA),
                "idxB": _wrap_idx(idxB),
                "relB": _wrap_rel(relB),
                "degE": degE[r * EO : (r + 1) * EO],
                "degV": degV[r * NO : (r + 1) * NO],
            }
        )
    return meta, per_core


def _build(meta):
    nchA = meta["nchA"]
    nchB = meta["nchB"]
    nch2 = meta["nch2"]
    LA = sum(a + b for a, b in zip(nchA, nchB)) * 128
    LB = sum(nch2) * 128
    NCHA_TOT = LA // 128
    NCHB_TOT = LB // 128

    nc = bacc.Bacc("TRN2", target_bir_lowering=False, debug=False, num_devices=R,
                   num_swdge_queues=4)

    # ---- kernel I/O ----
    xsh = nc.declare_dram_parameter("xsh", [NO, C], f32, isOutput=False)
    w_in = nc.declare_dram_parameter("w_in", [C, C], f32, isOutput=False)
    w1 = nc.declare_dram_parameter("w1", [C, C], f32, isOutput=False)
    w2a = nc.declare_dram_parameter("w2a", [C, C], f32, isOutput=False)
    w2b = nc.declare_dram_parameter("w2b", [C, C], f32, isOutput=False)
    w3h = nc.declare_dram_parameter("w3h", [C, C], f32, isOutput=False)
    b_in = nc.declare_dram_parameter("b_in", [C, 1], f32, isOutput=False)
    b2d = nc.declare_dram_parameter("b2", [C, 1], f32, isOutput=False)
    b3d = nc.declare_dram_parameter("b3", [C, 1], f32, isOutput=False)
    b1e_d = nc.declare_dram_parameter("b1e", [C, EO], f32, isOutput=False)
    dvrep_d = nc.declare_dram_parameter("dvrep", [C, NO], f32, isOutput=False)
    idxA_d = nc.declare_dram_parameter("idxA", [128, LA // 16], i16, isOutput=False)
    relA_d = nc.declare_dram_parameter("relA", [128, NCHA_TOT], f16, isOutput=False)
    idxB_d = nc.declare_dram_parameter("idxB", [128, LB // 16], i16, isOutput=False)
    relB_d = nc.declare_dram_parameter("relB", [128, NCHB_TOT], f16, isOutput=False)
    xout = nc.declare_dram_parameter("xout", [NO, C], f32, isOutput=True)

    # ---- internal DRAM ----
    agx_in = [nc.dram_tensor(f"agx_in{l}", [NO, C], f16) for l in range(N_LAYERS)]
    x_full = [nc.dram_tensor(f"x_full{l}", [N, C], f16, addr_space="Shared")
              for l in range(N_LAYERS)]
    agxe_in = [nc.dram_tensor(f"agxe_in{l}", [EO, C], f16) for l in range(N_LAYERS)]
    xe_full = [nc.dram_tensor(f"xe_full{l}", [M, C], f16, addr_space="Shared")
               for l in range(N_LAYERS)]

    rg = [list(range(R))]
    qrr = [0]

    def next_q():
        q = qrr[0]
        qrr[0] = (q + 1) % 4
        return q

    SPLIT_CHUNKS = 16  # max chunks (128 idxs each) per dma_gather call

    with tile.TileContext(nc) as tc:
        with (
            tc.tile_pool(name="const", bufs=1) as cp,
            tc.tile_pool(name="work", bufs=2) as wp,
            tc.tile_pool(name="ptiles", bufs=3) as pp,
            tc.tile_pool(name="psA", bufs=2, space="PSUM") as psA,
            tc.tile_pool(name="psB", bufs=2, space="PSUM") as psB,
            tc.tile_pool(name="psC", bufs=2, space="PSUM") as psC,
            tc.tile_pool(name="psD", bufs=2, space="PSUM") as psD,
        ):
            # ---------- persistent tiles ----------
            W_IN = cp.tile([C, C], f32)
            W1 = cp.tile([C, C], f32)
            W2A = cp.tile([C, C], f32)
            W2B = cp.tile([C, C], f32)
            W3H = cp.tile([C, C], f32)
            BIN = cp.tile([C, 1], f32)
            B2 = cp.tile([C, 1], f32)
            B3 = cp.tile([C, 1], f32)
            IDXA = cp.tile([128, LA // 16], i16)
            IDXB = cp.tile([128, LB // 16], i16)
            RELA = cp.tile([128, NCHA_TOT], f16)
            RELB = cp.tile([128, NCHB_TOT], f16)
            XFM = cp.tile([C, NO], f32)
            X0B = cp.tile([C, NO], f32)
            IOTAF = cp.tile([128, 128], f32)
            IOTA16 = cp.tile([128, 128], f16)
            PIDX = cp.tile([128, 1], f32)
            IDENT = cp.tile([128, 128], f32)

            for t, d in [
                (W_IN, w_in), (W1, w1), (W2A, w2a), (W2B, w2b), (W3H, w3h),
                (BIN, b_in), (B2, b2d), (B3, b3d),
                (IDXA, idxA_d), (IDXB, idxB_d), (RELA, relA_d), (RELB, relB_d),
            ]:
                nc.sync.dma_start(t[:], d[:])

            nc.gpsimd.iota(IOTAF[:], [[1, 128]], channel_multiplier=0,
                           allow_small_or_imprecise_dtypes=True)
            nc.gpsimd.iota(PIDX[:], [[1, 1]], channel_multiplier=1,
                           allow_small_or_imprecise_dtypes=True)
            nc.vector.tensor_scalar(IDENT[:], IOTAF[:], PIDX[:], None,
                                    mybir.AluOpType.is_equal)
            nc.vector.tensor_copy(IOTA16[:], IOTAF[:])


            def split_gather(gt, col0, nch_tot, src_ap, idx_tile, slot0):
                """One logical gather as ceil(nch_tot/SPLIT_CHUNKS) calls on
                rotating SWDGE queues (parallel descriptor gen)."""
                done = 0
                while done < nch_tot:
                    step = min(SPLIT_CHUNKS, nch_tot - done)
                    sl = slot0 + done * 128
                    nc.gpsimd.dma_gather(
                        out_ap=gt[:, col0 + done : col0 + done + step, :],
                        in_ap=src_ap,
                        idxs_ap=idx_tile[:, sl // 16 : (sl + step * 128) // 16],
                        num_idxs=step * 128, num_idxs_reg=step * 128,
                        elem_size=C, single_packet=False, queue_num=next_q(),
                    )
                    done += step

            def pgen(rel_tile, c0, nch):
                """P[p, c, j] = (rel[p, c0+c] == j), one DVE op."""
                P = pp.tile([128, max(nch, 1), 128], f16, tag="P")
                nc.vector.tensor_tensor(
                    P[:, :nch, :],
                    rel_tile[:, c0 : c0 + nch].unsqueeze(2)
                        .broadcast_to([128, nch, 128]),
                    IOTA16[:].unsqueeze(1).broadcast_to([128, nch, 128]),
                    mybir.AluOpType.is_equal)
                return P

            # ---------- prologue: x = relu(x @ W_in + b_in) ----------
            for n0 in range(0, NO, 512):
                nn = min(512, NO - n0)
                nsc = (nn + 127) // 128
                xin = wp.tile([128, 512], f32, tag="xin")
                for si in range(nsc):
                    ns = min(128, nn - si * 128)
                    nc.sync.dma_start(xin[:ns, si * 128 : si * 128 + C],
                                      xsh[n0 + si * 128 : n0 + si * 128 + ns, :])
                ptr = psD.tile([128, 512], f32, tag="tr")
                for si in range(nsc):
                    ns = min(128, nn - si * 128)
                    nc.tensor.transpose(ptr[:, si * 128 : si * 128 + ns],
                                        xin[:ns, si * 128 : si * 128 + C],
                                        IDENT[:ns, :ns])
                xT = wp.tile([C, 512], f32, tag="xT")
                nc.vector.tensor_copy(xT[:, :nn], ptr[:, :nn])
                pmm = psB.tile([C, 512], f32, tag="mmA")
                nc.tensor.matmul(pmm[:, :nn], W_IN[:], xT[:, :nn])
                nc.scalar.activation(XFM[:, n0 : n0 + nn], pmm[:, :nn],
                                     mybir.ActivationFunctionType.Relu,
                                     bias=BIN[:, :1])
                # X0B = x0 + deg_V * b2  (phase-B restart + bias, pre-folded)
                dvt = wp.tile([C, 512], f32, tag="dvt")
                nc.sync.dma_start(dvt[:, :nn], dvrep_d[:, n0 : n0 + nn])
                tmp = wp.tile([C, 512], f32, tag="xdeg")
                nc.vector.tensor_scalar(tmp[:, :nn], dvt[:, :nn], B2[:, :1],
                                        None, mybir.AluOpType.mult)
                nc.vector.tensor_tensor(X0B[:, n0 : n0 + nn], tmp[:, :nn],
                                        XFM[:, n0 : n0 + nn],
                                        mybir.AluOpType.add)
                # row-major f16 copy (gather source)
                ptr2 = psD.tile([128, 512], f32, tag="tr")
                xrm = wp.tile([128, 512], f16, tag="xrm")
                for si in range(nsc):
                    ns = min(128, nn - si * 128)
                    blk = slice(si * 128, si * 128 + C)
                    nc.tensor.transpose(
                        ptr2[:ns, blk],
                        XFM[:, n0 + si * 128 : n0 + si * 128 + ns], IDENT[:])
                    nc.vector.tensor_copy(xrm[:ns, blk], ptr2[:ns, blk])
                    nc.sync.dma_start(
                        agx_in[0][n0 + si * 128 : n0 + si * 128 + ns, :],
                        xrm[:ns, blk])
            nc.gpsimd.collective_compute(
                "AllGather", mybir.AluOpType.bypass, replica_groups=rg,
                ins=[agx_in[0][:]], outs=[x_full[0][:]],
            )

            # ---------- conv layers ----------
            for l in range(N_LAYERS):
                xf = x_full[l]
                # ---- phase A: Xe = (segsum_E x[V]) @ W1 + b1*degE ----
                colA = 0
                slotA = 0
                for grp in GROUPS_A:
                    e0g = grp[0] * 128
                    ne_g = min(128 * len(grp), EO - e0g)
                    na_g = sum(nchA[s] for s in grp)
                    nb_g = sum(nchB[s] for s in grp)
                    ntot = na_g + nb_g
                    gt = wp.tile([128, ntot, C], f16, tag="gath")
                    if na_g > 0:
                        split_gather(gt, 0, na_g, xf[:], IDXA, slotA)
                    if nb_g > 0:
                        split_gather(gt, na_g, nb_g, xf[SPLIT:, :], IDXA,
                                     slotA + na_g * 128)
                    slotA += ntot * 128
                    Pg = pgen(RELA, colA, ntot)
                    ps = psA.tile([C, 512], f32, tag="seg")
                    aoff = 0
                    boff = na_g
                    for si, s in enumerate(grp):
                        seq = (list(range(aoff, aoff + nchA[s]))
                               + list(range(boff, boff + nchB[s])))
                        aoff += nchA[s]
                        boff += nchB[s]
                        dst = ps[:, si * 128 : (si + 1) * 128]
                        for j, cc in enumerate(seq):
                            nc.tensor.matmul(dst, gt[:, cc, :], Pg[:, cc, :],
                                             start=(j == 0),
                                             stop=(j == len(seq) - 1))
                    colA += ntot
                    gsb = wp.tile([C, 512], f32, tag="gsb")
                    nc.vector.tensor_copy(gsb[:, :ne_g], ps[:, :ne_g])
                    b1t = wp.tile([C, 512], f32, tag="dvt")
                    nc.sync.dma_start(b1t[:, :ne_g], b1e_d[:, e0g : e0g + ne_g])
                    pxe = psB.tile([C, 512], f32, tag="mmA")
                    nc.tensor.matmul(pxe[:, :ne_g], W1[:], gsb[:, :ne_g])
                    xesb = wp.tile([C, 512], f32, tag="xesb")
                    nc.vector.tensor_tensor(xesb[:, :ne_g], pxe[:, :ne_g],
                                            b1t[:, :ne_g], mybir.AluOpType.add)
                    ptr = psD.tile([128, 512], f32, tag="tr")
                    xerm = wp.tile([128, 512], f16, tag="xrm")
                    for si in range(len(grp)):
                        ns = min(128, ne_g - si * 128)
                        if ns <= 0:
                            break
                        blk = slice(si * 128, si * 128 + C)
                        nc.tensor.transpose(
                            ptr[:ns, blk],
                            xesb[:, si * 128 : si * 128 + ns], IDENT[:])
                        nc.vector.tensor_copy(xerm[:ns, blk], ptr[:ns, blk])
                        nc.sync.dma_start(
                            agxe_in[l][e0g + si * 128 : e0g + si * 128 + ns, :],
                            xerm[:ns, blk])
                nc.gpsimd.collective_compute(
                    "AllGather", mybir.AluOpType.bypass, replica_groups=rg,
                    ins=[agxe_in[l][:]], outs=[xe_full[l][:]],
                )

                # ---- phase B ----
                last = l == N_LAYERS - 1
                colB = 0
                slotB = 0
                for grp in GROUPS_B:
                    n0g = grp[0] * 128
                    nn_g = min(128 * len(grp), NO - n0g)
                    nch_g = sum(nch2[s] for s in grp)
                    gt = wp.tile([128, nch_g, C], f16, tag="gath")
                    split_gather(gt, 0, nch_g, xe_full[l][:], IDXB, slotB)
                    slotB += nch_g * 128
                    Pg = pgen(RELB, colB, nch_g)
                    ps = psA.tile([C, 512], f32, tag="seg")
                    off = 0
                    for si, s in enumerate(grp):
                        dst = ps[:, si * 128 : (si + 1) * 128]
                        nchs = nch2[s]
                        for j in range(nchs):
                            nc.tensor.matmul(dst, gt[:, off + j, :],
                                             Pg[:, off + j, :],
                                             start=(j == 0),
                                             stop=(j == nchs - 1))
                        off += nchs
                    colB += nch_g
                    ysb = wp.tile([C, 512], f32, tag="gsb")
                    nc.vector.tensor_copy(ysb[:, :nn_g], ps[:, :nn_g])
                    dvt = wp.tile([C, 512], f32, tag="dvt")
                    nc.sync.dma_start(dvt[:, :nn_g], dvrep_d[:, n0g : n0g + nn_g])
                    xdeg = wp.tile([C, 512], f32, tag="xdeg")
                    nc.vector.tensor_tensor(xdeg[:, :nn_g],
                                            XFM[:, n0g : n0g + nn_g],
                                            dvt[:, :nn_g],
                                            mybir.AluOpType.mult)
                    pab = psB.tile([C, 512], f32, tag="mmA")
                    nc.tensor.matmul(pab[:, :nn_g], W2A[:], xdeg[:, :nn_g],
                                     start=True, stop=False)
                    nc.tensor.matmul(pab[:, :nn_g], W2B[:], ysb[:, :nn_g],
                                     start=False, stop=True)
                    xmid = wp.tile([C, 512], f32, tag="xesb")
                    nc.vector.tensor_tensor(xmid[:, :nn_g], pab[:, :nn_g],
                                            X0B[:, n0g : n0g + nn_g],
                                            mybir.AluOpType.add)
                    pc = psC.tile([C, 512], f32, tag="out")
                    nc.tensor.matmul(pc[:, :nn_g], W3H[:], xmid[:, :nn_g])
                    nc.scalar.activation(XFM[:, n0g : n0g + nn_g], pc[:, :nn_g],
                                         mybir.ActivationFunctionType.Relu,
                                         bias=B3[:, :1])
                    ptr = psD.tile([128, 512], f32, tag="tr")
                    xrm = wp.tile([128, 512], f32 if last else f16,
                                  tag="xrmf" if last else "xrm")
                    dstt = xout if last else agx_in[l + 1]
                    for si in range(len(grp)):
                        ns = min(128, nn_g - si * 128)
                        if ns <= 0:
                            break
                        blk = slice(si * 128, si * 128 + C)
                        nc.tensor.transpose(
                            ptr[:ns, blk],
                            XFM[:, n0g + si * 128 : n0g + si * 128 + ns],
                            IDENT[:])
                        nc.vector.tensor_copy(xrm[:ns, blk], ptr[:ns, blk])
                        nc.sync.dma_start(
                            dstt[n0g + si * 128 : n0g + si * 128 + ns, :],
                            xrm[:ns, blk])
                if not last:
                    nc.gpsimd.collective_compute(
                        "AllGather", mybir.AluOpType.bypass, replica_groups=rg,
                        ins=[agx_in[l + 1][:]], outs=[x_full[l + 1][:]],
                    )
    nc.compile()
    return nc


def _get_program(V, E):
    key = (hash(V.tobytes()), hash(E.tobytes()))
    if key not in _cache:
        meta, per_core = _prepare(V, E)
        nc = _build(meta)
        _cache[key] = (nc, per_core)
    return _cache[key]


def run(trace=False, trace_kwargs=None, **inputs):
    x = np.ascontiguousarray(np.asarray(inputs["x"], dtype=np.float32))
    V = np.asarray(inputs["V"]).astype(np.int64)
    E = np.asarray(inputs["E"]).astype(np.int64)
    W_in = np.ascontiguousarray(np.asarray(inputs["W_in"], np.float32))
    b_in = np.asarray(inputs["b_in"], np.float32).reshape(C, 1)
    W1 = np.ascontiguousarray(np.asarray(inputs["W1"], np.float32))
    b1 = np.asarray(inputs["b1"], np.float32).reshape(C)
    W2 = np.asarray(inputs["W2"], np.float32)
    b2 = np.asarray(inputs["b2"], np.float32).reshape(C, 1)
    W3 = np.asarray(inputs["W3"], np.float32)
    b3 = np.asarray(inputs["b3"], np.float32).reshape(C, 1)
    W2a = np.ascontiguousarray(W2[:C])
    W2b = np.ascontiguousarray(W2[C:])
    W3h = np.ascontiguousarray((1.0 - ALPHA) * W3)
    # note: (1-a)*Xv + a*x0 = (1-a)*(Xv + x0) since a = 0.5

    nc, per_core = _get_program(V, E)

    in_maps = []
    for r in range(R):
        pc = per_core[r]
        b1e = np.ascontiguousarray(np.outer(b1, pc["degE"]).astype(np.float32))
        dvrep = np.ascontiguousarray(
            np.broadcast_to(pc["degV"], (C, NO)).astype(np.float32))
        in_maps.append({
            "xsh": x[r * NO : (r + 1) * NO],
            "w_in": W_in, "w1": W1, "w2a": W2a, "w2b": W2b, "w3h": W3h,
            "b_in": b_in, "b2": b2, "b3": b3,
            "b1e": b1e, "dvrep": dvrep,
            "idxA": pc["idxA"], "relA": pc["relA"],
            "idxB": pc["idxB"], "relB": pc["relB"],
        })
    res = run_bass_kernel_spmd(nc, in_maps, list(range(R)), trace=trace,
                               **(trace_kwargs or {}))
    out = np.concatenate([res.results[r]["xout"] for r in range(R)], axis=0)
    return out, res


def kernel(**inputs):
    out, _ = run(**inputs)
    return out
